# revision 18
# baseline (speedup 1.0000x reference)
import numpy as np
import ml_dtypes

import concourse.bacc as bacc
import concourse.tile as tile
from concourse import mybir

# NIMSCrossEntropyLoss: loss = [sum_px lse_c(p) - sum_px p[tgt]]/4, S=-1.
# v21: v19 + W as fp8 in P0 pad, fp8 exp outputs, (p-1) host shift (partition c*32+r), quarter-granular streaming:
#   P1..P4 fp8 [128,1024] pred quarters   (sync/HWDGE, small first chunk)
#   T1..T4 fp8 [128,1024] target-replica  (gpsimd/SWDGE after cvec+W)
#   W bf16 [128,32] ones-blocks; cvec f32 [128,1] = p//32
# ACT: exp per quarter -> ln(psumA), ln(psumB).
# PE: 4 concurrent tile-positioned matmuls per half into psumA/psumB.
# DVE: 4 quarter mask-dot stts (is_equal vs cvec, mult, accum).

N_CORES = 8
P = 128
C = 4
N_BATCH = 4
FD = 4096
QD = FD // 4      # 1024 per quarter

FP8 = mybir.dt.float8e4
BF16 = mybir.dt.bfloat16
F32 = mybir.dt.float32

_PATCHED = False


def _patch_act_tables():
    global _PATCHED
    if _PATCHED:
        return
    import concourse.hw_specs as hw_specs
    real = hw_specs.get_activation_tables
    Exp = mybir.ActivationFunctionType.Exp
    Ln = mybir.ActivationFunctionType.Ln

    def patched(arch):
        out = {}
        for name, fns in dict(real(arch)).items():
            if name != "natural_log_exp_and_others":
                fns = fns - {Exp, Ln}
            out[name] = fns
        return out

    bacc.get_activation_tables = patched
    _PATCHED = True


def build_nc(finalize=True):
    """out [P, 6] f32: cols 0..3 = quarter mask-dot accums,
    cols 4..5 = ln accums (halves)."""
    _patch_act_tables()
    nc = bacc.Bacc("TRN2", target_bir_lowering=False, debug=False)
    inP0 = nc.dram_tensor("inpP0", (P, QD + 40), FP8, kind="ExternalInput").ap()
    inP = [nc.dram_tensor(f"inpP{i}", (P, QD), FP8, kind="ExternalInput").ap()
           for i in range(1, 4)]
    inT = [nc.dram_tensor(f"inpT{i}", (P, QD), FP8, kind="ExternalInput").ap()
           for i in range(4)]
    out = nc.dram_tensor("out", (P, 6), F32, kind="ExternalOutput").ap()

    Exp = mybir.ActivationFunctionType.Exp
    Ln = mybir.ActivationFunctionType.Ln

    with tile.TileContext(nc) as tc:
        with tc.tile_pool(name="w", bufs=1) as w, \
             tc.tile_pool(name="ps", bufs=1, space="PSUM") as ps:
            tP0 = w.tile([P, QD + 40], FP8, name="tP0")
            tP = [tP0[:, 0:QD]] + \
                 [w.tile([P, QD], FP8, name=f"tP{i}") for i in range(1, 4)]
            zed = tP0[:, QD:QD + 1]
            tC = tP0[:, QD + 1:QD + 2]
            tT = [w.tile([P, QD], FP8, name=f"tT{i}") for i in range(4)]
            tW = tP0[:, QD + 2:QD + 34]

            nc.sync.dma_start(out=tP0, in_=inP0)
            for i in range(1, 4):
                nc.sync.dma_start(out=tP[i], in_=inP[i - 1])
            # SWDGE queue carries only T-quarters; W rides in P0's pad
            # (fp8 ones-blocks -> fp8 matmul, probe-proven).
            for i in range(4):
                nc.gpsimd.dma_start(out=tT[i], in_=inT[i])

            res = w.tile([P, 6], F32, name="res")
            e = [w.tile([P, QD], FP8, name=f"e{i}") for i in range(4)]
            psumA = ps.tile([P, 512], F32, name="psumA")
            psumB = ps.tile([P, 512], F32, name="psumB")

            for i in range(4):
                nc.scalar.activation(out=e[i], in_=tP[i], func=Exp, bias=zed)

            # channel-sum matmuls: half A = quarters 0,1; half B = 2,3.
            # Each quarter contributes two 512-col col-group matmuls.
            for h, pt in ((0, psumA), (2, psumB)):
                for j in range(4):
                    qi = h + j // 2
                    sl = (j % 2) * 512
                    nc.tensor.matmul(out=pt[j * 32:(j + 1) * 32, :],
                                     lhsT=tW, rhs=e[qi][:, sl:sl + 512],
                                     start=True, stop=True,
                                     tile_position=(0, j * 32))

            scr = w.tile([P, QD], BF16, name="scr")
            for i in range(4):
                nc.vector.scalar_tensor_tensor(
                    out=scr, in0=tT[i], scalar=tC, in1=tP[i],
                    op0=mybir.AluOpType.is_equal, op1=mybir.AluOpType.mult,
                    accum_out=res[:, i:i + 1],
                )

            lnout = w.tile([P, 512], BF16, name="lnout")
            nc.scalar.activation(out=lnout, in_=psumA, func=Ln, bias=zed,
                                 accum_out=res[:, 4:5])
            nc.scalar.activation(out=lnout, in_=psumB, func=Ln, bias=zed,
                                 accum_out=res[:, 5:6])

            nc.sync.dma_start(out=out, in_=res)
    for func in nc.m.functions:
        for block in func.blocks:
            block.instructions = [
                i for i in block.instructions
                if type(i).__name__ != "InstMemset"
            ]
    if finalize:
        nc.finalize()
    return nc


_NC_CACHE = {}


def _get_nc():
    if "nc" not in _NC_CACHE:
        _NC_CACHE["nc"] = build_nc()
    return _NC_CACHE["nc"]


def prep_inputs(preds, targets):
    p = np.asarray(preds)[:, -1]
    t = np.asarray(targets)[:, -1]
    arr = np.transpose(p, (1, 0, 2, 3)).reshape(C, N_CORES, 32, FD)
    arr = (arr - 1.0).astype(ml_dtypes.float8_e4m3)
    tf = t.reshape(N_CORES, 32, FD).astype(ml_dtypes.float8_e4m3)

    maps = []
    for k in range(N_CORES):
        pb = arr[:, k].reshape(P, FD)
        trep = np.tile(tf[k], (4, 1))
        m = {}
        p0 = np.zeros((P, QD + 40), dtype=ml_dtypes.float8_e4m3)
        p0[:, 0:QD] = pb[:, 0:QD]
        p0[:, QD + 1] = (np.arange(P) // 32).astype(ml_dtypes.float8_e4m3)
        for pp in range(P):
            p0[pp, QD + 2 + pp % 32] = 1.0
        m["inpP0"] = p0
        for i in range(4):
            if i:
                m[f"inpP{i}"] = np.ascontiguousarray(pb[:, i * QD:(i + 1) * QD])
            m[f"inpT{i}"] = np.ascontiguousarray(trep[:, i * QD:(i + 1) * QD])
        maps.append(m)
    return maps


def reduce_outputs(results):
    total = 0.0
    for d in results:
        o = d["out"].astype(np.float64)
        total += float(o[:, 4:6].sum() - o[:, 0:4].sum())
    return np.float32(total / N_BATCH)


def kernel(preds, targets, _trace=False, _trace_kwargs=None):
    from concourse.bass_utils import run_bass_kernel_spmd

    in_maps = prep_inputs(preds, targets)
    nc = _get_nc()
    r = run_bass_kernel_spmd(
        nc, in_maps, core_ids=list(range(N_CORES)),
        trace=_trace, **(_trace_kwargs or {}),
    )
    kernel.last_run = r
    return reduce_outputs(r.results)


kernel.last_run = None
